# revision 25
# baseline (speedup 1.0000x reference)
"""Causal single-head attention (B=4, S=2048, D=1024) on 8 TRN2 NeuronCores.

Sharding: data-parallel, 2 cores per batch element. The 16 query blocks
(128 rows each) of a batch are split between its two cores in a
triangle-balanced pattern (A: blocks 0,3,4,7,8,11,12,15 / B: the rest) so
both cores see the same multiset of padded causal key-range lengths
(256, 512, ..., 2048) — every core runs an identical instruction graph,
per-core structure lives entirely in the numpy-side input sharding.

Per core (uniform graph), all matmuls bf16 with fp32 PSUM accumulation:
  phase 1ab (fused, one pass over x^T): K^T = wk @ x^T and V = x @ wv^T
  phase 1c: Q^T = wq @ xq^T (et-outer, streamed wq panels)
  phase 2, per query slot s (8 slots of 128 q-rows, key range 256*(s+1)):
    scores -> additive causal mask on the last 256 key cols (fed as data)
    -> exp (no max-subtraction: |scores|/32 <= ~11) with accumulated row
    sums -> PE-transpose of probs -> probs^T @ V -> scale by 1/rowsum -> out.

Inputs are pre-cast to bf16 and pre-paneled host-side so every DMA is
contiguous; DMA emission order == HWDGE FIFO order == first-use order.

kernel(x, wq, wk, wv) takes full inputs, returns the full [4,2048,1024] output.
"""
import ml_dtypes
import numpy as np

import concourse.bacc as bacc
import concourse.mybir as mybir
import concourse.tile as tile
from concourse.bass_utils import run_bass_kernel_spmd

F32 = mybir.dt.float32
BF16 = mybir.dt.bfloat16
AX = mybir.AxisListType
AF = mybir.ActivationFunctionType

P = 128
B, S, D = 4, 2048, 1024
NQ = 1024            # query rows per core
BLOCKS_A = [0, 3, 4, 7, 8, 11, 12, 15]
BLOCKS_B = [1, 2, 5, 6, 9, 10, 13, 14]
NEG = -1e30
SCALE = 1.0 / 32.0   # 1/sqrt(D)

_CACHE = {}


def _build():
    nc = bacc.Bacc("TRN2", target_bir_lowering=False, debug=False)
    # x chunks pre-paneled host-side: [ck, p, dt, s] = x[ck*512+s, dt*128+p]
    xTP_d = nc.declare_dram_parameter("xTP", [4, P, 8, 512], BF16, isOutput=False)
    xqTP_d = nc.declare_dram_parameter("xqTP", [2, P, 8, 512], BF16, isOutput=False)
    # lhsT weights pre-paneled host-side: [et, p, dt, es] with
    # w_panel[et, p, dt, es] = w.T[dt*128+p, et*128+es]
    wqp_d = nc.declare_dram_parameter("wqp", [8, P, 8, P], BF16, isOutput=False)
    wkp_d = nc.declare_dram_parameter("wkp", [8, P, 8, P], BF16, isOutput=False)
    wvT_d = nc.declare_dram_parameter("wvT", [D, D], BF16, isOutput=False)
    mb_d = nc.declare_dram_parameter("maskb", [P, 8, 256], F32, isOutput=False)
    out_d = nc.declare_dram_parameter("out", [NQ, D], F32, isOutput=True)

    def wv_tiled(lo, hi):
        # [ (t p), e ] -> [ p, t, lo:hi ]
        return wvT_d.ap().rearrange("(t p) f -> p t f", p=P)[:, :, lo:hi]

    with tile.TileContext(nc) as tc:
        with (
            tc.tile_pool(name="store", bufs=1) as store,
            tc.tile_pool(name="wpool", bufs=2) as wpool,
            tc.tile_pool(name="wqpool", bufs=3) as wqpool,
            tc.tile_pool(name="xpool", bufs=4) as xpool,
            tc.tile_pool(name="small", bufs=4) as smallp,
            tc.tile_pool(name="outp", bufs=3) as outp,
            tc.tile_pool(name="ps_a", bufs=1, space="PSUM") as psA,
            tc.tile_pool(name="ps_b", bufs=6, space="PSUM") as psB,
        ):
            KT = store.tile([P, 8, S], BF16)    # [e%128, e//128, key]
            V = store.tile([P, 16, D], BF16)    # [k%128, k//128, e]
            QT = store.tile([P, 8, NQ], BF16)   # [e%128, e//128, q]

            # DMA emission order == HWDGE FIFO order: emit strictly in order of
            # first use so early matmuls aren't stuck behind later transfers.
            wk = wpool.tile([P, 8, 8, P], BF16, tag="w")   # [p, et, dt, es]
            nc.sync.dma_start(wk[:, 0], wkp_d.ap()[0])
            xts = []
            xt = xpool.tile([P, 8, 512], BF16, tag="x")
            nc.sync.dma_start(xt[:, 0:4], xTP_d.ap()[0][:, 0:4])
            nc.sync.dma_start(xt[:, 4:8], xTP_d.ap()[0][:, 4:8])
            xts.append(xt)
            for et in range(1, 8):
                nc.sync.dma_start(wk[:, et], wkp_d.ap()[et])
            xt = xpool.tile([P, 8, 512], BF16, tag="x")
            nc.sync.dma_start(xt[:], xTP_d.ap()[1])
            xts.append(xt)
            wv = wpool.tile([P, 8, D], BF16, tag="w")      # [p, dt, e] rhs layout
            nc.sync.dma_start(wv[:, :, 0:512], wv_tiled(0, 512))
            nc.sync.dma_start(wv[:, :, 512:1024], wv_tiled(512, 1024))

            # ---- phase 1ab fused: KT = wk @ x^T, V = x @ wv^T ----
            for ck in range(4):
                if ck < 2:
                    xt = xts[ck]
                else:
                    xt = xpool.tile([P, 8, 512], BF16, tag="x")
                    nc.sync.dma_start(xt[:], xTP_d.ap()[ck])
                for et in range(8):
                    ps = psB.tile([P, 512], F32, tag="pb")
                    for dt in range(8):
                        nc.tensor.matmul(
                            ps[:],
                            wk[:, et, dt, :],
                            xt[:, dt, :],
                            start=(dt == 0),
                            stop=(dt == 7),
                        )
                    nc.any.tensor_copy(KT[:, et, ck * 512:(ck + 1) * 512], ps[:])
                for sub in range(4):
                    for ev in range(2):
                        ps = psB.tile([P, 512], F32, tag="pb")
                        for dt in range(8):
                            nc.tensor.matmul(
                                ps[:],
                                xt[:, dt, sub * P:(sub + 1) * P],
                                wv[:, dt, ev * 512:(ev + 1) * 512],
                                start=(dt == 0),
                                stop=(dt == 7),
                            )
                        nc.any.tensor_copy(
                            V[:, ck * 4 + sub, ev * 512:(ev + 1) * 512], ps[:]
                        )

            # ---- phase 1c: QT = wq @ xq^T (et-outer, streamed wq panels) ----
            xqs = []
            for cq in range(2):
                xq = xpool.tile([P, 8, 512], BF16, tag="x")
                nc.sync.dma_start(xq[:], xqTP_d.ap()[cq])
                xqs.append(xq)
            wq_panels = []
            for et in range(8):
                wqp = wqpool.tile([P, 8, P], BF16, tag="wq")
                nc.sync.dma_start(wqp[:], wqp_d.ap()[et])
                wq_panels.append(wqp)
            maskt = store.tile([P, 8, 256], F32)
            nc.sync.dma_start(maskt[:], mb_d.ap())
            for et in range(8):
                wqp = wq_panels[et]
                for cq in range(2):
                    ps = psB.tile([P, 512], F32, tag="pb")
                    for dt in range(8):
                        nc.tensor.matmul(
                            ps[:],
                            wqp[:, dt, :],
                            xqs[cq][:, dt, :],
                            start=(dt == 0),
                            stop=(dt == 7),
                        )
                    nc.any.tensor_copy(QT[:, et, cq * 512:(cq + 1) * 512], ps[:])

            # ---- phase 2: attention per slot (largest first), software-
            # pipelined one slot deep: slot s's probs are transposed via the
            # XBAR DMA engine (ACT ring) while slot s-1's scores run on the PE,
            # and slot s's AV matmuls are emitted after slot s-1's scores so
            # the transpose latency is hidden. ----
            def emit_av(s, probsT, rec, ltiles):
                for ev in range(2):
                    pav = psB.tile([P, 512], F32, tag="pb")
                    for t in range(ltiles):
                        nc.tensor.matmul(
                            pav[:],
                            probsT[:, t, :],
                            V[:, t, ev * 512:(ev + 1) * 512],
                            start=(t == 0),
                            stop=(t == ltiles - 1),
                        )
                    ot = outp.tile([P, 512], F32, tag="out")
                    nc.scalar.activation(ot[:], pav[:], AF.Copy, scale=rec[:])
                    nc.sync.dma_start(
                        out_d.ap()[s * P:(s + 1) * P, ev * 512:(ev + 1) * 512],
                        ot[:],
                    )

            pending = None
            for s in range(7, -1, -1):
                ltiles = 2 * (s + 1)           # 128-wide key tiles
                keys = 256 * (s + 1)
                n512 = (s + 1) // 2            # full 512-wide chunks
                rem = (s + 1) % 2              # one trailing 256-wide chunk?
                nch = n512 + rem
                probs = xpool.tile([P, S], BF16, tag="x")
                probsT = xpool.tile([P, 16, P], BF16, tag="x")
                sums = smallp.tile([P, 8], F32, tag="sums")
                for c in range(nch):
                    is_rem = rem and c == nch - 1
                    w = 256 if is_rem else 512
                    lo = c * 512
                    if is_rem:
                        ps = psA.tile([P, 256], F32, tag="pa")
                    else:
                        ps = psB.tile([P, 512], F32, tag="pb")
                    for dt in range(8):
                        nc.tensor.matmul(
                            ps[:],
                            QT[:, dt, s * P:(s + 1) * P],
                            KT[:, dt, lo:lo + w],
                            start=(dt == 0),
                            stop=(dt == 7),
                        )
                    if lo + w == keys:   # mask the last 256 key columns
                        nc.vector.tensor_add(
                            ps[:, w - 256:w], ps[:, w - 256:w], maskt[:, s, :]
                        )
                    nc.scalar.activation(
                        probs[:, lo:lo + w],
                        ps[:],
                        AF.Exp,
                        scale=SCALE,
                        accum_out=sums[:, c:c + 1],
                    )
                    nc.scalar.dma_start(
                        probsT[:, lo // P:(lo + w) // P, :],
                        probs[:, lo:lo + w],
                        transpose=True,
                    )
                den = smallp.tile([P, 1], F32, tag="den")
                nc.vector.reduce_sum(den[:], sums[:, :nch], axis=AX.X)
                rec = smallp.tile([P, 1], F32, tag="rec")
                nc.vector.reciprocal(rec[:], den[:])

                if pending is not None:
                    emit_av(*pending)
                pending = (s, probsT, rec, ltiles)
            emit_av(*pending)
    nc.compile()
    return nc


def _make_masks():
    masks = []
    for blocks in (BLOCKS_A, BLOCKS_B):
        m = np.zeros((P, 8, 256), np.float32)
        for s, j in enumerate(blocks):
            q = j * P + np.arange(P)[:, None]
            k = 256 * s + np.arange(256)[None, :]
            m[:, s, :] = np.where(k <= q, 0.0, NEG)
        masks.append(m)
    return masks


def _bf16(a):
    return np.ascontiguousarray(a.astype(ml_dtypes.bfloat16))


def _panelize(wT):
    # wT: [D, D] = [dt*128+p, et*128+es] -> [et, p, dt, es]
    return _bf16(wT.reshape(8, P, 8, P).transpose(2, 1, 0, 3))


def _chunk_panels(rows, nck):
    # [nck*512, 1024] -> [ck, p, dt, s] with [ck,p,dt,s] = rows[ck*512+s, dt*128+p]
    return _bf16(rows.reshape(nck, 512, 8, P).transpose(0, 3, 2, 1))


LAST_RESULT = None


def kernel(x, wq, wk, wv):
    global LAST_RESULT
    x = np.ascontiguousarray(np.asarray(x, dtype=np.float32))
    wq = np.asarray(wq, dtype=np.float32)
    wk = np.asarray(wk, dtype=np.float32)
    wv = np.asarray(wv, dtype=np.float32)

    if "nc" not in _CACHE:
        _CACHE["nc"] = _build()
        _CACHE["masks"] = _make_masks()
    nc = _CACHE["nc"]
    masks = _CACHE["masks"]

    wqp = _panelize(wq.T)
    wkp = _panelize(wk.T)
    wvT = _bf16(wv.T)

    in_maps = []
    for c in range(8):
        b, pat = divmod(c, 2)
        blocks = BLOCKS_A if pat == 0 else BLOCKS_B
        xb = x[b]
        xq = np.concatenate([xb[j * P:(j + 1) * P] for j in blocks], 0)
        in_maps.append(
            {
                "xTP": _chunk_panels(xb, 4),
                "xqTP": _chunk_panels(xq, 2),
                "wqp": wqp,
                "wkp": wkp,
                "wvT": wvT,
                "maskb": masks[pat],
            }
        )

    res = run_bass_kernel_spmd(nc, in_maps, core_ids=list(range(8)))
    LAST_RESULT = res

    out = np.empty((B, S, D), np.float32)
    for c in range(8):
        b, pat = divmod(c, 2)
        blocks = BLOCKS_A if pat == 0 else BLOCKS_B
        oc = res.results[c]["out"]
        for si, j in enumerate(blocks):
            out[b, j * P:(j + 1) * P] = oc[si * P:(si + 1) * P]
    return out


# revision 26
# speedup vs baseline: 1.0405x; 1.0405x over previous
"""Causal single-head attention (B=4, S=2048, D=1024) on 8 TRN2 NeuronCores.

Sharding: data-parallel, 2 cores per batch element. The 16 query blocks
(128 rows each) of a batch are split between its two cores in a
triangle-balanced pattern (A: blocks 0,3,4,7,8,11,12,15 / B: the rest) so
both cores see the same multiset of padded causal key-range lengths
(256, 512, ..., 2048) — every core runs an identical instruction graph,
per-core structure lives entirely in the numpy-side input sharding.

Per core (uniform graph), all matmuls bf16 with fp32 PSUM accumulation:
  phase 1ab (fused, one pass over x^T): K^T = wk @ x^T and V = x @ wv^T
  phase 1c: Q^T = wq @ xq^T (et-outer, streamed wq panels)
  phase 2, per query slot s (8 slots of 128 q-rows, key range 256*(s+1)):
    scores -> additive causal mask on the last 256 key cols (fed as data)
    -> exp (no max-subtraction: |scores|/32 <= ~11) with accumulated row
    sums -> PE-transpose of probs -> probs^T @ V -> scale by 1/rowsum -> out.

Inputs are pre-cast to bf16 and pre-paneled host-side so every DMA is
contiguous; DMA emission order == HWDGE FIFO order == first-use order.

kernel(x, wq, wk, wv) takes full inputs, returns the full [4,2048,1024] output.
"""
import ml_dtypes
import numpy as np

import concourse.bacc as bacc
import concourse.mybir as mybir
import concourse.tile as tile
from concourse.bass_utils import run_bass_kernel_spmd
from concourse.masks import make_identity

F32 = mybir.dt.float32
BF16 = mybir.dt.bfloat16
AX = mybir.AxisListType
AF = mybir.ActivationFunctionType

P = 128
B, S, D = 4, 2048, 1024
NQ = 1024            # query rows per core
BLOCKS_A = [0, 3, 4, 7, 8, 11, 12, 15]
BLOCKS_B = [1, 2, 5, 6, 9, 10, 13, 14]
NEG = -1e30
SCALE = 1.0 / 32.0   # 1/sqrt(D)

_CACHE = {}


def _build():
    nc = bacc.Bacc("TRN2", target_bir_lowering=False, debug=False)
    # x chunks pre-paneled host-side: [ck, p, dt, s] = x[ck*512+s, dt*128+p]
    xTP_d = nc.declare_dram_parameter("xTP", [4, P, 8, 512], BF16, isOutput=False)
    xqTP_d = nc.declare_dram_parameter("xqTP", [2, P, 8, 512], BF16, isOutput=False)
    # lhsT weights pre-paneled host-side: [et, p, dt, es] with
    # w_panel[et, p, dt, es] = w.T[dt*128+p, et*128+es]
    wqp_d = nc.declare_dram_parameter("wqp", [8, P, 8, P], BF16, isOutput=False)
    wkp_d = nc.declare_dram_parameter("wkp", [8, P, 8, P], BF16, isOutput=False)
    wvT_d = nc.declare_dram_parameter("wvT", [D, D], BF16, isOutput=False)
    mb_d = nc.declare_dram_parameter("maskb", [P, 8, 256], F32, isOutput=False)
    out_d = nc.declare_dram_parameter("out", [NQ, D], F32, isOutput=True)

    def wv_tiled(lo, hi):
        # [ (t p), e ] -> [ p, t, lo:hi ]
        return wvT_d.ap().rearrange("(t p) f -> p t f", p=P)[:, :, lo:hi]

    with tile.TileContext(nc) as tc:
        with (
            tc.tile_pool(name="store", bufs=1) as store,
            tc.tile_pool(name="wpool", bufs=2) as wpool,
            tc.tile_pool(name="wqpool", bufs=3) as wqpool,
            tc.tile_pool(name="xpool", bufs=4) as xpool,
            tc.tile_pool(name="small", bufs=4) as smallp,
            tc.tile_pool(name="outp", bufs=3) as outp,
            tc.tile_pool(name="ps_a", bufs=1, space="PSUM") as psA,
            tc.tile_pool(name="ps_b", bufs=5, space="PSUM") as psB,
            tc.tile_pool(name="ps_t", bufs=2, space="PSUM") as psT,
        ):
            ident = store.tile([P, P], BF16)
            make_identity(nc, ident[:])
            KT = store.tile([P, 8, S], BF16)    # [e%128, e//128, key]
            V = store.tile([P, 16, D], BF16)    # [k%128, k//128, e]
            QT = store.tile([P, 8, NQ], BF16)   # [e%128, e//128, q]

            # DMA emission order == HWDGE FIFO order: emit strictly in order of
            # first use so early matmuls aren't stuck behind later transfers.
            wk = wpool.tile([P, 8, 8, P], BF16, tag="w")   # [p, et, dt, es]
            nc.sync.dma_start(wk[:, 0], wkp_d.ap()[0])
            xts = []
            xt = xpool.tile([P, 8, 512], BF16, tag="x")
            nc.sync.dma_start(xt[:, 0:4], xTP_d.ap()[0][:, 0:4])
            nc.sync.dma_start(xt[:, 4:8], xTP_d.ap()[0][:, 4:8])
            xts.append(xt)
            for et in range(1, 8):
                nc.sync.dma_start(wk[:, et], wkp_d.ap()[et])
            xt = xpool.tile([P, 8, 512], BF16, tag="x")
            nc.sync.dma_start(xt[:], xTP_d.ap()[1])
            xts.append(xt)
            wv = wpool.tile([P, 8, D], BF16, tag="w")      # [p, dt, e] rhs layout
            nc.sync.dma_start(wv[:, :, 0:512], wv_tiled(0, 512))
            nc.sync.dma_start(wv[:, :, 512:1024], wv_tiled(512, 1024))

            # ---- phase 1ab fused: KT = wk @ x^T, V = x @ wv^T ----
            for ck in range(4):
                if ck < 2:
                    xt = xts[ck]
                else:
                    xt = xpool.tile([P, 8, 512], BF16, tag="x")
                    nc.sync.dma_start(xt[:], xTP_d.ap()[ck])
                for et in range(8):
                    ps = psB.tile([P, 512], F32, tag="pb")
                    for dt in range(8):
                        nc.tensor.matmul(
                            ps[:],
                            wk[:, et, dt, :],
                            xt[:, dt, :],
                            start=(dt == 0),
                            stop=(dt == 7),
                        )
                    nc.any.tensor_copy(KT[:, et, ck * 512:(ck + 1) * 512], ps[:])
                for sub in range(4):
                    for ev in range(2):
                        ps = psB.tile([P, 512], F32, tag="pb")
                        for dt in range(8):
                            nc.tensor.matmul(
                                ps[:],
                                xt[:, dt, sub * P:(sub + 1) * P],
                                wv[:, dt, ev * 512:(ev + 1) * 512],
                                start=(dt == 0),
                                stop=(dt == 7),
                            )
                        nc.any.tensor_copy(
                            V[:, ck * 4 + sub, ev * 512:(ev + 1) * 512], ps[:]
                        )

            # ---- phase 1c: QT = wq @ xq^T (et-outer, streamed wq panels) ----
            xqs = []
            for cq in range(2):
                xq = xpool.tile([P, 8, 512], BF16, tag="x")
                nc.sync.dma_start(xq[:], xqTP_d.ap()[cq])
                xqs.append(xq)
            wq_panels = []
            for et in range(8):
                wqp = wqpool.tile([P, 8, P], BF16, tag="wq")
                nc.sync.dma_start(wqp[:], wqp_d.ap()[et])
                wq_panels.append(wqp)
            maskt = store.tile([P, 8, 256], F32)
            nc.sync.dma_start(maskt[:], mb_d.ap())
            for et in range(8):
                wqp = wq_panels[et]
                for cq in range(2):
                    ps = psB.tile([P, 512], F32, tag="pb")
                    for dt in range(8):
                        nc.tensor.matmul(
                            ps[:],
                            wqp[:, dt, :],
                            xqs[cq][:, dt, :],
                            start=(dt == 0),
                            stop=(dt == 7),
                        )
                    nc.any.tensor_copy(QT[:, et, cq * 512:(cq + 1) * 512], ps[:])

            # ---- phase 2: attention per slot (largest first), software-
            # pipelined one slot deep: slot s's probs are transposed via the
            # XBAR DMA engine (ACT ring) while slot s-1's scores run on the PE,
            # and slot s's AV matmuls are emitted after slot s-1's scores so
            # the transpose latency is hidden. ----
            def emit_av(s, probsT, rec, ltiles):
                for ev in range(2):
                    pav = psB.tile([P, 512], F32, tag="pb")
                    for t in range(ltiles):
                        nc.tensor.matmul(
                            pav[:],
                            probsT[:, t, :],
                            V[:, t, ev * 512:(ev + 1) * 512],
                            start=(t == 0),
                            stop=(t == ltiles - 1),
                        )
                    ot = outp.tile([P, 512], F32, tag="out")
                    nc.scalar.activation(ot[:], pav[:], AF.Copy, scale=rec[:])
                    nc.sync.dma_start(
                        out_d.ap()[s * P:(s + 1) * P, ev * 512:(ev + 1) * 512],
                        ot[:],
                    )

            pending = None
            for s in range(7, -1, -1):
                ltiles = 2 * (s + 1)           # 128-wide key tiles
                keys = 256 * (s + 1)
                n512 = (s + 1) // 2            # full 512-wide chunks
                rem = (s + 1) % 2              # one trailing 256-wide chunk?
                nch = n512 + rem
                probs = xpool.tile([P, S], BF16, tag="x")
                probsT = xpool.tile([P, 16, P], BF16, tag="x")
                sums = smallp.tile([P, 8], F32, tag="sums")
                for c in range(nch):
                    is_rem = rem and c == nch - 1
                    w = 256 if is_rem else 512
                    lo = c * 512
                    if is_rem:
                        ps = psA.tile([P, 256], F32, tag="pa")
                    else:
                        ps = psB.tile([P, 512], F32, tag="pb")
                    for dt in range(8):
                        nc.tensor.matmul(
                            ps[:],
                            QT[:, dt, s * P:(s + 1) * P],
                            KT[:, dt, lo:lo + w],
                            start=(dt == 0),
                            stop=(dt == 7),
                        )
                    if lo + w == keys:   # mask the last 256 key columns
                        nc.vector.tensor_add(
                            ps[:, w - 256:w], ps[:, w - 256:w], maskt[:, s, :]
                        )
                    nc.scalar.activation(
                        probs[:, lo:lo + w],
                        ps[:],
                        AF.Exp,
                        scale=SCALE,
                        accum_out=sums[:, c:c + 1],
                    )
                    if s >= 2:
                        # XBAR (DMA-engine) transpose; latency hides behind the
                        # next slot's score matmuls (AV is emitted one slot late)
                        nc.scalar.dma_start(
                            probsT[:, lo // P:(lo + w) // P, :],
                            probs[:, lo:lo + w],
                            transpose=True,
                        )
                    else:
                        # pipeline runs dry at the tail: the last two (small)
                        # slots transpose on the PE to avoid DMA latency stalls
                        for t in range(lo // P, (lo + w) // P):
                            pt = psT.tile([P, P], BF16, tag="tr")
                            nc.tensor.transpose(
                                pt[:], probs[:, t * P:(t + 1) * P], ident[:]
                            )
                            nc.any.tensor_copy(probsT[:, t, :], pt[:])
                den = smallp.tile([P, 1], F32, tag="den")
                nc.vector.reduce_sum(den[:], sums[:, :nch], axis=AX.X)
                rec = smallp.tile([P, 1], F32, tag="rec")
                nc.vector.reciprocal(rec[:], den[:])

                if pending is not None:
                    emit_av(*pending)
                pending = (s, probsT, rec, ltiles)
            emit_av(*pending)
    nc.compile()
    return nc


def _make_masks():
    masks = []
    for blocks in (BLOCKS_A, BLOCKS_B):
        m = np.zeros((P, 8, 256), np.float32)
        for s, j in enumerate(blocks):
            q = j * P + np.arange(P)[:, None]
            k = 256 * s + np.arange(256)[None, :]
            m[:, s, :] = np.where(k <= q, 0.0, NEG)
        masks.append(m)
    return masks


def _bf16(a):
    return np.ascontiguousarray(a.astype(ml_dtypes.bfloat16))


def _panelize(wT):
    # wT: [D, D] = [dt*128+p, et*128+es] -> [et, p, dt, es]
    return _bf16(wT.reshape(8, P, 8, P).transpose(2, 1, 0, 3))


def _chunk_panels(rows, nck):
    # [nck*512, 1024] -> [ck, p, dt, s] with [ck,p,dt,s] = rows[ck*512+s, dt*128+p]
    return _bf16(rows.reshape(nck, 512, 8, P).transpose(0, 3, 2, 1))


LAST_RESULT = None


def kernel(x, wq, wk, wv):
    global LAST_RESULT
    x = np.ascontiguousarray(np.asarray(x, dtype=np.float32))
    wq = np.asarray(wq, dtype=np.float32)
    wk = np.asarray(wk, dtype=np.float32)
    wv = np.asarray(wv, dtype=np.float32)

    if "nc" not in _CACHE:
        _CACHE["nc"] = _build()
        _CACHE["masks"] = _make_masks()
    nc = _CACHE["nc"]
    masks = _CACHE["masks"]

    wqp = _panelize(wq.T)
    wkp = _panelize(wk.T)
    wvT = _bf16(wv.T)

    in_maps = []
    for c in range(8):
        b, pat = divmod(c, 2)
        blocks = BLOCKS_A if pat == 0 else BLOCKS_B
        xb = x[b]
        xq = np.concatenate([xb[j * P:(j + 1) * P] for j in blocks], 0)
        in_maps.append(
            {
                "xTP": _chunk_panels(xb, 4),
                "xqTP": _chunk_panels(xq, 2),
                "wqp": wqp,
                "wkp": wkp,
                "wvT": wvT,
                "maskb": masks[pat],
            }
        )

    res = run_bass_kernel_spmd(nc, in_maps, core_ids=list(range(8)))
    LAST_RESULT = res

    out = np.empty((B, S, D), np.float32)
    for c in range(8):
        b, pat = divmod(c, 2)
        blocks = BLOCKS_A if pat == 0 else BLOCKS_B
        oc = res.results[c]["out"]
        for si, j in enumerate(blocks):
            out[b, j * P:(j + 1) * P] = oc[si * P:(si + 1) * P]
    return out


# revision 27
# speedup vs baseline: 1.0405x; 1.0000x over previous
"""Causal single-head attention (B=4, S=2048, D=1024) on 8 TRN2 NeuronCores.

Sharding: data-parallel, 2 cores per batch element. The 16 query blocks
(128 rows each) of a batch are split between its two cores in a
triangle-balanced pattern (A: blocks 0,3,4,7,8,11,12,15 / B: the rest) so
both cores see the same multiset of padded causal key-range lengths
(256, 512, ..., 2048) — every core runs an identical instruction graph,
per-core structure lives entirely in the numpy-side input sharding.

Per core (uniform graph), all matmuls bf16 with fp32 PSUM accumulation:
  phase 1ab (fused, one pass over x^T): K^T = wk @ x^T and V = x @ wv^T
  phase 1c: Q^T = wq @ xq^T (et-outer, streamed wq panels)
  phase 2, per query slot s (8 slots of 128 q-rows, key range 256*(s+1)):
    scores -> additive causal mask on the last 256 key cols (fed as data)
    -> exp (no max-subtraction: |scores|/32 <= ~11) with accumulated row
    sums -> transpose probs (XBAR DMA for the 6 big slots, pipelined one
    slot deep so latency hides; PE for the last 2) -> probs^T @ V -> scale
    by 1/rowsum -> out.

Inputs are pre-cast to bf16 and pre-paneled host-side so every DMA is
contiguous; DMA emission order == HWDGE FIFO order == first-use order.

kernel(x, wq, wk, wv) takes full inputs, returns the full [4,2048,1024] output.
"""
import ml_dtypes
import numpy as np

import concourse.bacc as bacc
import concourse.mybir as mybir
import concourse.tile as tile
from concourse.bass_utils import run_bass_kernel_spmd
from concourse.masks import make_identity

F32 = mybir.dt.float32
BF16 = mybir.dt.bfloat16
AX = mybir.AxisListType
AF = mybir.ActivationFunctionType

P = 128
B, S, D = 4, 2048, 1024
NQ = 1024            # query rows per core
BLOCKS_A = [0, 3, 4, 7, 8, 11, 12, 15]
BLOCKS_B = [1, 2, 5, 6, 9, 10, 13, 14]
NEG = -1e30
SCALE = 1.0 / 32.0   # 1/sqrt(D)

_CACHE = {}


def _build():
    nc = bacc.Bacc("TRN2", target_bir_lowering=False, debug=False)
    # x chunks pre-paneled host-side: [ck, p, dt, s] = x[ck*512+s, dt*128+p]
    xTP_d = nc.declare_dram_parameter("xTP", [4, P, 8, 512], BF16, isOutput=False)
    xqTP_d = nc.declare_dram_parameter("xqTP", [2, P, 8, 512], BF16, isOutput=False)
    # lhsT weights pre-paneled host-side: [et, p, dt, es] with
    # w_panel[et, p, dt, es] = w.T[dt*128+p, et*128+es]
    wqp_d = nc.declare_dram_parameter("wqp", [8, P, 8, P], BF16, isOutput=False)
    wkp_d = nc.declare_dram_parameter("wkp", [8, P, 8, P], BF16, isOutput=False)
    wvT_d = nc.declare_dram_parameter("wvT", [D, D], BF16, isOutput=False)
    mb_d = nc.declare_dram_parameter("maskb", [P, 8, 256], F32, isOutput=False)
    out_d = nc.declare_dram_parameter("out", [NQ, D], F32, isOutput=True)

    def wv_tiled(lo, hi):
        # [ (t p), e ] -> [ p, t, lo:hi ]
        return wvT_d.ap().rearrange("(t p) f -> p t f", p=P)[:, :, lo:hi]

    with tile.TileContext(nc) as tc:
        with (
            tc.tile_pool(name="store", bufs=1) as store,
            tc.tile_pool(name="wpool", bufs=2) as wpool,
            tc.tile_pool(name="wqpool", bufs=3) as wqpool,
            tc.tile_pool(name="xpool", bufs=4) as xpool,
            tc.tile_pool(name="small", bufs=4) as smallp,
            tc.tile_pool(name="outp", bufs=3) as outp,
            tc.tile_pool(name="ps_a", bufs=1, space="PSUM") as psA,
            tc.tile_pool(name="ps_b", bufs=5, space="PSUM") as psB,
            tc.tile_pool(name="ps_t", bufs=2, space="PSUM") as psT,
        ):
            ident = store.tile([P, P], BF16)
            make_identity(nc, ident[:])
            KT = store.tile([P, 8, S], BF16)    # [e%128, e//128, key]
            V = store.tile([P, 16, D], BF16)    # [k%128, k//128, e]
            QT = store.tile([P, 8, NQ], BF16)   # [e%128, e//128, q]

            # DMA emission order == HWDGE FIFO order: emit strictly in order of
            # first use so early matmuls aren't stuck behind later transfers.
            wk = wpool.tile([P, 8, 8, P], BF16, tag="w")   # [p, et, dt, es]
            nc.sync.dma_start(wk[:, 0], wkp_d.ap()[0])
            xts = []
            xt = xpool.tile([P, 8, 512], BF16, tag="x")
            nc.sync.dma_start(xt[:, 0:4], xTP_d.ap()[0][:, 0:4])
            nc.sync.dma_start(xt[:, 4:8], xTP_d.ap()[0][:, 4:8])
            xts.append(xt)
            for et in range(1, 8):
                nc.sync.dma_start(wk[:, et], wkp_d.ap()[et])
            xt = xpool.tile([P, 8, 512], BF16, tag="x")
            nc.sync.dma_start(xt[:], xTP_d.ap()[1])
            xts.append(xt)
            wv = wpool.tile([P, 8, D], BF16, tag="w")      # [p, dt, e] rhs layout
            nc.sync.dma_start(wv[:, :, 0:512], wv_tiled(0, 512))
            nc.sync.dma_start(wv[:, :, 512:1024], wv_tiled(512, 1024))

            # ---- phase 1ab fused: KT = wk @ x^T, V = x @ wv^T ----
            for ck in range(4):
                if ck < 2:
                    xt = xts[ck]
                else:
                    xt = xpool.tile([P, 8, 512], BF16, tag="x")
                    nc.sync.dma_start(xt[:], xTP_d.ap()[ck])
                for et in range(8):
                    ps = psB.tile([P, 512], F32, tag="pb")
                    for dt in range(8):
                        nc.tensor.matmul(
                            ps[:],
                            wk[:, et, dt, :],
                            xt[:, dt, :],
                            start=(dt == 0),
                            stop=(dt == 7),
                        )
                    nc.any.tensor_copy(KT[:, et, ck * 512:(ck + 1) * 512], ps[:])
                for sub in range(4):
                    for ev in range(2):
                        ps = psB.tile([P, 512], F32, tag="pb")
                        for dt in range(8):
                            nc.tensor.matmul(
                                ps[:],
                                xt[:, dt, sub * P:(sub + 1) * P],
                                wv[:, dt, ev * 512:(ev + 1) * 512],
                                start=(dt == 0),
                                stop=(dt == 7),
                            )
                        nc.any.tensor_copy(
                            V[:, ck * 4 + sub, ev * 512:(ev + 1) * 512], ps[:]
                        )

            # ---- phase 1c: QT = wq @ xq^T (et-outer, streamed wq panels) ----
            xqs = []
            for cq in range(2):
                xq = xpool.tile([P, 8, 512], BF16, tag="x")
                nc.sync.dma_start(xq[:], xqTP_d.ap()[cq])
                xqs.append(xq)
            wq_panels = []
            for et in range(8):
                wqp = wqpool.tile([P, 8, P], BF16, tag="wq")
                nc.sync.dma_start(wqp[:], wqp_d.ap()[et])
                wq_panels.append(wqp)
            maskt = store.tile([P, 8, 256], F32)
            nc.sync.dma_start(maskt[:], mb_d.ap())
            for et in range(8):
                wqp = wq_panels[et]
                for cq in range(2):
                    ps = psB.tile([P, 512], F32, tag="pb")
                    for dt in range(8):
                        nc.tensor.matmul(
                            ps[:],
                            wqp[:, dt, :],
                            xqs[cq][:, dt, :],
                            start=(dt == 0),
                            stop=(dt == 7),
                        )
                    nc.any.tensor_copy(QT[:, et, cq * 512:(cq + 1) * 512], ps[:])

            # ---- phase 2: attention per slot (largest first), software-
            # pipelined one slot deep: slot s's probs are transposed via the
            # XBAR DMA engine (ACT ring) while slot s-1's scores run on the PE,
            # and slot s's AV matmuls are emitted after slot s-1's scores so
            # the transpose latency is hidden. ----
            def emit_av(s, probsT, rec, ltiles):
                for ev in range(2):
                    pav = psB.tile([P, 512], F32, tag="pb")
                    for t in range(ltiles):
                        nc.tensor.matmul(
                            pav[:],
                            probsT[:, t, :],
                            V[:, t, ev * 512:(ev + 1) * 512],
                            start=(t == 0),
                            stop=(t == ltiles - 1),
                        )
                    ot = outp.tile([P, 512], F32, tag="out")
                    nc.scalar.activation(ot[:], pav[:], AF.Copy, scale=rec[:])
                    nc.sync.dma_start(
                        out_d.ap()[s * P:(s + 1) * P, ev * 512:(ev + 1) * 512],
                        ot[:],
                    )

            pending = None
            for s in range(7, -1, -1):
                ltiles = 2 * (s + 1)           # 128-wide key tiles
                keys = 256 * (s + 1)
                n512 = (s + 1) // 2            # full 512-wide chunks
                rem = (s + 1) % 2              # one trailing 256-wide chunk?
                nch = n512 + rem
                probs = xpool.tile([P, S], BF16, tag="x")
                probsT = xpool.tile([P, 16, P], BF16, tag="x")
                sums = smallp.tile([P, 8], F32, tag="sums")
                for c in range(nch):
                    is_rem = rem and c == nch - 1
                    w = 256 if is_rem else 512
                    lo = c * 512
                    if is_rem:
                        ps = psA.tile([P, 256], F32, tag="pa")
                    else:
                        ps = psB.tile([P, 512], F32, tag="pb")
                    for dt in range(8):
                        nc.tensor.matmul(
                            ps[:],
                            QT[:, dt, s * P:(s + 1) * P],
                            KT[:, dt, lo:lo + w],
                            start=(dt == 0),
                            stop=(dt == 7),
                        )
                    if lo + w == keys:   # mask the last 256 key columns
                        nc.vector.tensor_add(
                            ps[:, w - 256:w], ps[:, w - 256:w], maskt[:, s, :]
                        )
                    nc.scalar.activation(
                        probs[:, lo:lo + w],
                        ps[:],
                        AF.Exp,
                        scale=SCALE,
                        accum_out=sums[:, c:c + 1],
                    )
                    if s >= 2:
                        # XBAR (DMA-engine) transpose; latency hides behind the
                        # next slot's score matmuls (AV is emitted one slot late)
                        nc.scalar.dma_start(
                            probsT[:, lo // P:(lo + w) // P, :],
                            probs[:, lo:lo + w],
                            transpose=True,
                        )
                    else:
                        # pipeline runs dry at the tail: the last two (small)
                        # slots transpose on the PE to avoid DMA latency stalls
                        for t in range(lo // P, (lo + w) // P):
                            pt = psT.tile([P, P], BF16, tag="tr")
                            nc.tensor.transpose(
                                pt[:], probs[:, t * P:(t + 1) * P], ident[:]
                            )
                            nc.any.tensor_copy(probsT[:, t, :], pt[:])
                den = smallp.tile([P, 1], F32, tag="den")
                nc.vector.reduce_sum(den[:], sums[:, :nch], axis=AX.X)
                rec = smallp.tile([P, 1], F32, tag="rec")
                nc.vector.reciprocal(rec[:], den[:])

                if pending is not None:
                    emit_av(*pending)
                pending = (s, probsT, rec, ltiles)
            emit_av(*pending)
    nc.compile()
    return nc


def _make_masks():
    masks = []
    for blocks in (BLOCKS_A, BLOCKS_B):
        m = np.zeros((P, 8, 256), np.float32)
        for s, j in enumerate(blocks):
            q = j * P + np.arange(P)[:, None]
            k = 256 * s + np.arange(256)[None, :]
            m[:, s, :] = np.where(k <= q, 0.0, NEG)
        masks.append(m)
    return masks


def _bf16(a):
    return np.ascontiguousarray(a.astype(ml_dtypes.bfloat16))


def _panelize(wT):
    # wT: [D, D] = [dt*128+p, et*128+es] -> [et, p, dt, es]
    return _bf16(wT.reshape(8, P, 8, P).transpose(2, 1, 0, 3))


def _chunk_panels(rows, nck):
    # [nck*512, 1024] -> [ck, p, dt, s] with [ck,p,dt,s] = rows[ck*512+s, dt*128+p]
    return _bf16(rows.reshape(nck, 512, 8, P).transpose(0, 3, 2, 1))


LAST_RESULT = None


def kernel(x, wq, wk, wv):
    global LAST_RESULT
    x = np.ascontiguousarray(np.asarray(x, dtype=np.float32))
    wq = np.asarray(wq, dtype=np.float32)
    wk = np.asarray(wk, dtype=np.float32)
    wv = np.asarray(wv, dtype=np.float32)

    if "nc" not in _CACHE:
        _CACHE["nc"] = _build()
        _CACHE["masks"] = _make_masks()
    nc = _CACHE["nc"]
    masks = _CACHE["masks"]

    wqp = _panelize(wq.T)
    wkp = _panelize(wk.T)
    wvT = _bf16(wv.T)

    in_maps = []
    for c in range(8):
        b, pat = divmod(c, 2)
        blocks = BLOCKS_A if pat == 0 else BLOCKS_B
        xb = x[b]
        xq = np.concatenate([xb[j * P:(j + 1) * P] for j in blocks], 0)
        in_maps.append(
            {
                "xTP": _chunk_panels(xb, 4),
                "xqTP": _chunk_panels(xq, 2),
                "wqp": wqp,
                "wkp": wkp,
                "wvT": wvT,
                "maskb": masks[pat],
            }
        )

    res = run_bass_kernel_spmd(nc, in_maps, core_ids=list(range(8)))
    LAST_RESULT = res

    out = np.empty((B, S, D), np.float32)
    for c in range(8):
        b, pat = divmod(c, 2)
        blocks = BLOCKS_A if pat == 0 else BLOCKS_B
        oc = res.results[c]["out"]
        for si, j in enumerate(blocks):
            out[b, j * P:(j + 1) * P] = oc[si * P:(si + 1) * P]
    return out
